# revision 8
# baseline (speedup 1.0000x reference)
"""ConsensusAttention Trainium2 kernel.

Reference computation (per (b, l) of levels (8, 1024, 6, 512)):
    k = levels * rsqrt(max(sum(levels^2), 1e-12))          # GLOBAL l2 scale
    sim[b,l,i,j] = (q_i . k_j) / sqrt(512)
    sim diag <- -0.0005 ; sim[dist(i,j) > 2] <- -FLT_MAX   # 32x32 grid, radius 2
    out = softmax(sim) @ v

Strategy: data-parallel over batch - 8 cores, one batch each, single NEFF.

Per (b, l) the attention matrix is banded (13 grid-neighbor diagonals,
offsets in [-66, 66]); each 128-query tile attends inside a 384-wide aligned
key slab. Logits are tiny (|sim| < ~0.01): softmax needs no max subtraction,
masked entries get an additive -1e38 so exp underflows to exactly 0. The band
is symmetric (S = X X^T, symmetric mask), so the transposed A-blocks needed
by the A @ V matmul are exactly the A-blocks computed by the neighboring
query tiles - A is never transposed on chip.

Pipeline per core: stream the 12.6 MB shard in; per arriving tile compute
sum-of-squares partials (ScalarE Square+accum) and a float16 cast; transpose
the fp16 data with xbar DMA (X^T feeds the QK matmuls); AllReduce a single
(128,1) sum-of-squares vector to get the global scale (rsqrt via Ln+Exp -
same ACT table set as the softmax exp); then per (l, t): QK fp16 matmuls
-> +mask (DVE) -> exp with row-sum accumulator (ScalarE) -> A (fp16)
-> A @ V fp16 matmuls -> normalize during the PSUM->SBUF copy -> store.
float16 keeps matmul operands on the fast weight-load path; the output
error vs the fp32 reference stays ~1e-4 of scale.
"""

import math
import os

import numpy as np

B, N, L, D = 8, 1024, 6, 512
GRID = 32            # 32x32 patch grid, row-major
RADIUS2 = 4          # radius 2.0 squared
SELF_LOGIT = -0.0005
SELF_W = float(np.exp(np.float32(SELF_LOGIT)))
L2_EPS = 1e-12
P = 128              # partitions
NT = N // P          # 8 query tiles per (b, l)
KD = D // P          # 4 contraction chunks
W = 3 * P            # 384-wide key slab (aligned tiles t-1, t, t+1)
NEG = -1e38

_cache: dict = {}


def _masks():
    """Additive mask (NT, P, W): 0 on valid off-diagonal neighbors, -1e38 on
    diagonal / non-neighbors / out-of-range."""
    i = np.arange(N)
    hi, wi = i // GRID, i % GRID
    m = np.full((NT, P, W), NEG, np.float32)
    for t in range(NT):
        iq = t * P + np.arange(P)
        jk = (t - 1) * P + np.arange(W)
        valid = (jk >= 0) & (jk < N)
        jj = np.clip(jk, 0, N - 1)
        dh = hi[iq][:, None] - hi[jj][None, :]
        dw = wi[iq][:, None] - wi[jj][None, :]
        keep = (dh * dh + dw * dw <= RADIUS2) & valid[None, :] & (iq[:, None] != jj[None, :])
        m[t][keep] = 0.0
    return m


def _build_nc():
    import concourse.mybir as mybir
    import concourse.tile as tile
    from concourse import bacc

    F = mybir.dt.float32
    FR = mybir.dt.float32r
    H = mybir.dt.float16
    AF = mybir.ActivationFunctionType
    ADD = mybir.AluOpType.add

    nc = bacc.Bacc("TRN2", target_bir_lowering=False, debug=False,
                   enable_asserts=True, num_devices=8)
    x = nc.dram_tensor("x", [N, L, D], F, kind="ExternalInput").ap()
    m2 = nc.dram_tensor("m2", [NT, P, W], F, kind="ExternalInput").ap()
    di = nc.dram_tensor("di", [P, P], H, kind="ExternalInput").ap()
    ones = nc.dram_tensor("ones", [P, P], F, kind="ExternalInput").ap()
    o = nc.dram_tensor("o", [N, L, D], F, kind="ExternalOutput").ap()

    xr = x.rearrange("(j p) l d -> p j l d", p=P)
    orr = o.rearrange("(j p) l d -> p j l d", p=P)

    with tile.TileContext(nc) as tc:
        with tc.tile_pool(name="const", bufs=1) as cst, \
             tc.tile_pool(name="xin", bufs=3) as xin, \
             tc.tile_pool(name="pers", bufs=1) as pers, \
             tc.tile_pool(name="xt", bufs=4) as xtp, \
             tc.tile_pool(name="ab", bufs=2) as abp, \
             tc.tile_pool(name="col", bufs=2) as colp, \
             tc.tile_pool(name="outp", bufs=6) as outp, \
             tc.tile_pool(name="dram", bufs=1, space="DRAM") as dram, \
             tc.tile_pool(name="ps_s", bufs=3, space="PSUM") as ps_s, \
             tc.tile_pool(name="ps_o", bufs=2, space="PSUM") as ps_o, \
             tc.tile_pool(name="ps_r", bufs=1, space="PSUM") as ps_r:

            m2_sb = cst.tile([P, NT, W], F)
            nc.sync.dma_start(m2_sb[:], m2.rearrange("t p w -> p t w"))
            di_sb = cst.tile([P, P], H)
            nc.sync.dma_start(di_sb[:], di)
            ones_sb = cst.tile([P, P], F)
            nc.sync.dma_start(ones_sb[:], ones)

            # ---- stream shard in: squares + fp16 cast per token tile ----
            xh = pers.tile([P, L, NT, D], H)       # fp16 shard, 6.3 MB
            part = pers.tile([P, 2 * NT], F)       # per-half-tile sumsq partials
            HLF = L * D // 2
            for j in range(NT):
                xt_in = xin.tile([P, L * D], F, tag="x")
                nc.sync.dma_start(xt_in[:], xr[:, j, :, :])
                for h in range(2):
                    sq = xin.tile([P, HLF], F, tag="sq")
                    nc.scalar.activation(sq[:], xt_in[:, h * HLF:(h + 1) * HLF],
                                         AF.Square, bias=0.0, scale=1.0,
                                         accum_out=part[:, 2 * j + h:2 * j + h + 1])
                nc.gpsimd.tensor_copy(
                    xh[:, :, j, :],
                    xt_in[:].rearrange("p (l d) -> p l d", l=L))

            # ---- global scale c = rsqrt(max(ssq, eps)) / sqrt(D) ----
            p128 = colp.tile([P, 1], F, tag="p128")
            nc.vector.reduce_sum(p128[:], part[:], axis=mybir.AxisListType.X)
            pr = ps_r.tile([P, 1], F)
            nc.tensor.matmul(pr[:], ones_sb[:], p128[:], start=True, stop=True)
            ss_sb = colp.tile([P, 1], F, tag="ss")
            nc.vector.tensor_copy(ss_sb[:], pr[:])
            cc_in = dram.tile([P, 1], F)
            cc_out = dram.tile([P, 1], F)
            nc.sync.dma_start(cc_in[:], ss_sb[:])
            nc.gpsimd.collective_compute(
                "AllReduce", ADD, replica_groups=[list(range(8))],
                ins=[cc_in.opt()], outs=[cc_out.opt()])
            gs = colp.tile([P, 1], F, tag="gs")
            nc.sync.dma_start(gs[:], cc_out[:])
            nc.vector.tensor_scalar_max(gs[:], gs[:], float(L2_EPS))
            lns = colp.tile([P, 1], F, tag="lns")
            nc.scalar.activation(lns[:], gs[:], AF.Ln, bias=0.0, scale=float(D))
            c_sb = colp.tile([P, 1], F, tag="c")
            nc.scalar.activation(c_sb[:], lns[:], AF.Exp, bias=0.0, scale=-0.5)

            # ---- attention per level ----
            for l in range(L):
                # X^T via xbar DMA transpose: xt_sb[p, kd, tok] = X[tok, kd*128+p]
                xt_sb = xtp.tile([P, KD, N], H, tag="xt")
                for j in range(NT):
                    nc.sync.dma_start_transpose(xt_sb[:, :, j * P:(j + 1) * P],
                                                xh[:, l, j, :])

                a_sb = abp.tile([P, NT, W], H, tag="a")
                dsum = colp.tile([P, NT], F, tag="dsum")
                for t in range(NT):
                    lo, hi = max(t - 1, 0), min(t + 2, NT)
                    c0, c1 = (lo - t + 1) * P, (hi - t + 1) * P
                    s_ps = ps_s.tile([P, W], F, tag="sps")
                    for kd in range(KD):
                        nc.tensor.matmul(s_ps[:, c0:c1],
                                         xt_sb[:, kd, t * P:(t + 1) * P],
                                         xt_sb[:, kd, lo * P:hi * P],
                                         start=(kd == 0), stop=(kd == KD - 1))
                    nc.vector.tensor_tensor(s_ps[:, c0:c1], s_ps[:, c0:c1],
                                            m2_sb[:, t, c0:c1], ADD)
                    nc.scalar.activation(a_sb[:, t, c0:c1], s_ps[:, c0:c1], AF.Exp,
                                         bias=0.0, scale=c_sb[:],
                                         accum_out=dsum[:, t:t + 1])
                    # self-attention weight onto the diagonal of block (t, t)
                    nc.vector.tensor_tensor(a_sb[:, t, P:2 * P], a_sb[:, t, P:2 * P],
                                            di_sb[:], ADD)

                denom = colp.tile([P, NT], F, tag="den")
                nc.vector.tensor_scalar_add(denom[:], dsum[:], SELF_W)
                recip = colp.tile([P, NT], F, tag="rec")
                nc.vector.reciprocal(recip[:], denom[:])

                for t in range(NT):
                    o_ps = ps_o.tile([P, D], F, tag="ops")
                    ks = [k for k in (t - 1, t, t + 1) if 0 <= k < NT]
                    for r, k in enumerate(ks):
                        # block(k, t): A rows of query-tile k, key-tile t
                        nc.tensor.matmul(o_ps[:],
                                         a_sb[:, k, (t - k + 1) * P:(t - k + 2) * P],
                                         xh[:, l, k, :],
                                         start=(r == 0), stop=(r == len(ks) - 1))
                    out_sb = outp.tile([P, D], F, tag="o")
                    if t % 2 == 0:
                        nc.scalar.activation(out_sb[:], o_ps[:], AF.Copy,
                                             bias=0.0, scale=recip[:, t:t + 1])
                    else:
                        nc.vector.tensor_scalar_mul(out_sb[:], o_ps[:],
                                                    recip[:, t:t + 1])
                    nc.sync.dma_start(orr[:, t, l, :], out_sb[:])
    nc.compile()
    return nc


def _get_nc():
    if "nc" not in _cache:
        _cache["nc"] = _build_nc()
    return _cache["nc"]


def _consts():
    if "consts" not in _cache:
        _cache["consts"] = {
            "m2": _masks(),
            "di": (np.float16(SELF_W) * np.eye(P)).astype(np.float16),
            "ones": np.ones((P, P), np.float32),
        }
    return _cache["consts"]


def _install_ntff_hook():
    """The agent image's antenv package lacks axon_hooks; recreate the NTFF
    profile hook (ctypes into libaxon_pjrt.so) and register it so
    run_bass_kernel_spmd(trace=True) can capture profiles. Only used by the
    local test harness (KERNEL_TRACE=1); never on the default path."""
    if _cache.get("hook_installed"):
        return
    import contextlib
    import ctypes
    import sys
    import types

    so_path = "/opt/axon/libaxon_pjrt.so"
    lib = ctypes.CDLL(so_path)
    lib.axon_start_nrt_profile.argtypes = [ctypes.POINTER(ctypes.c_int64), ctypes.c_size_t]
    lib.axon_start_nrt_profile.restype = ctypes.c_int64
    lib.axon_stop_nrt_profile.argtypes = [ctypes.c_char_p]
    lib.axon_stop_nrt_profile.restype = ctypes.c_int64

    @contextlib.contextmanager
    def _hook(output_dir, device_ids):
        import jax
        jax.devices()
        if device_ids:
            ids = (ctypes.c_int64 * len(device_ids))(*device_ids)
            rc = lib.axon_start_nrt_profile(ids, len(device_ids))
        else:
            rc = lib.axon_start_nrt_profile(None, 0)
        if rc != 0:
            raise RuntimeError(f"axon_start_nrt_profile rc={rc}")
        try:
            yield
        finally:
            n = lib.axon_stop_nrt_profile(str(output_dir).encode())
            print(f"ntff profile: {n} file(s) written to {output_dir}", file=sys.stderr)

    mod = types.ModuleType("antenv.axon_hooks")
    mod.get_axon_ntff_profile_hook = lambda: _hook
    mod.set_axon_ntff_profile_hook = lambda h: None
    import antenv
    antenv.axon_hooks = mod
    sys.modules["antenv.axon_hooks"] = mod
    _cache["hook_installed"] = True


last_exec_time_ns = {"norm": None, "attn": None}


def kernel(levels: np.ndarray) -> np.ndarray:
    from concourse.bass_utils import run_bass_kernel_spmd

    assert levels.shape == (B, N, L, D) and levels.dtype == np.float32
    nc = _get_nc()
    trace = os.environ.get("KERNEL_TRACE", "0") == "1"
    if trace:
        try:
            _install_ntff_hook()
        except Exception as e:
            print(f"ntff hook unavailable ({e}); tracing disabled")
            trace = False
    cores = list(range(8))

    consts = _consts()
    in_maps = [{"x": np.ascontiguousarray(levels[b]), "m2": consts["m2"],
                "di": consts["di"], "ones": consts["ones"]} for b in range(B)]
    if trace:
        try:
            res = run_bass_kernel_spmd(nc, in_maps, core_ids=cores, trace=True)
        except Exception as e:
            print(f"traced run failed ({e}); retrying untraced")
            res = run_bass_kernel_spmd(nc, in_maps, core_ids=cores)
    else:
        res = run_bass_kernel_spmd(nc, in_maps, core_ids=cores)
    last_exec_time_ns["attn"] = res.exec_time_ns
    last_exec_time_ns["norm"] = 0

    return np.stack([r["o"] for r in res.results], axis=0)


# revision 11
# speedup vs baseline: 1.2139x; 1.2139x over previous
"""ConsensusAttention Trainium2 kernel.

Reference computation (per (b, l) of levels (8, 1024, 6, 512)):
    k = levels * rsqrt(max(sum(levels^2), 1e-12))          # GLOBAL l2 scale
    sim[b,l,i,j] = (q_i . k_j) / sqrt(512)
    sim diag <- -0.0005 ; sim[dist(i,j) > 2] <- -FLT_MAX   # 32x32 grid, radius 2
    out = softmax(sim) @ v

Strategy: data-parallel over batch - 8 cores, one batch each, single NEFF.

Per (b, l) the attention matrix is banded (13 grid-neighbor diagonals,
offsets in [-66, 66]); each 128-query tile attends inside a 384-wide aligned
key slab. Logits are tiny (|sim| < ~0.01): softmax needs no max subtraction,
masked entries get an additive -1e38 so exp underflows to exactly 0. The band
is symmetric (S = X X^T, symmetric mask), so the transposed A-blocks needed
by the A @ V matmul are exactly the A-blocks computed by the neighboring
query tiles - A is never transposed on chip.

Pipeline per core: stream the 12.6 MB shard in; per arriving tile compute
sum-of-squares partials (ScalarE Square+accum) and a float16 cast; transpose
the fp16 data with xbar DMA (X^T feeds the QK matmuls); AllReduce a single
(128,1) sum-of-squares vector to get the global scale (rsqrt via Ln+Exp -
same ACT table set as the softmax exp); then per (l, t): QK fp16 matmuls
-> +mask (DVE) -> exp with row-sum accumulator (ScalarE) -> A (fp16)
-> A @ V fp16 matmuls -> normalize during the PSUM->SBUF copy -> store.
float16 keeps matmul operands on the fast weight-load path; the output
error vs the fp32 reference stays ~1e-4 of scale.
"""

import math
import os

import numpy as np

B, N, L, D = 8, 1024, 6, 512
GRID = 32            # 32x32 patch grid, row-major
RADIUS2 = 4          # radius 2.0 squared
SELF_LOGIT = -0.0005
SELF_W = float(np.exp(np.float32(SELF_LOGIT)))
L2_EPS = 1e-12
P = 128              # partitions
NT = N // P          # 8 query tiles per (b, l)
KD = D // P          # 4 contraction chunks
W = 3 * P            # 384-wide key slab (aligned tiles t-1, t, t+1)
NEG = -1e38

_cache: dict = {}


def _masks():
    """Additive mask (NT, P, W): 0 on valid off-diagonal neighbors, -1e38 on
    diagonal / non-neighbors / out-of-range."""
    i = np.arange(N)
    hi, wi = i // GRID, i % GRID
    m = np.full((NT, P, W), NEG, np.float32)
    for t in range(NT):
        iq = t * P + np.arange(P)
        jk = (t - 1) * P + np.arange(W)
        valid = (jk >= 0) & (jk < N)
        jj = np.clip(jk, 0, N - 1)
        dh = hi[iq][:, None] - hi[jj][None, :]
        dw = wi[iq][:, None] - wi[jj][None, :]
        keep = (dh * dh + dw * dw <= RADIUS2) & valid[None, :] & (iq[:, None] != jj[None, :])
        m[t][keep] = 0.0
    return m


def _build_nc():
    import concourse.mybir as mybir
    import concourse.tile as tile
    from concourse import bacc

    F = mybir.dt.float32
    FR = mybir.dt.float32r
    H = mybir.dt.float16
    AF = mybir.ActivationFunctionType
    ADD = mybir.AluOpType.add

    nc = bacc.Bacc("TRN2", target_bir_lowering=False, debug=False,
                   enable_asserts=True, num_devices=8)
    x = nc.dram_tensor("x", [N, L, D], F, kind="ExternalInput").ap()
    m2 = nc.dram_tensor("m2", [NT, P, W], F, kind="ExternalInput").ap()
    di = nc.dram_tensor("di", [P, P], H, kind="ExternalInput").ap()
    ones = nc.dram_tensor("ones", [P, P], F, kind="ExternalInput").ap()
    o = nc.dram_tensor("o", [N, L, D], F, kind="ExternalOutput").ap()

    xr = x.rearrange("(j p) l d -> p j l d", p=P)
    orr = o.rearrange("(j p) l d -> p j l d", p=P)

    with tile.TileContext(nc) as tc:
        with tc.tile_pool(name="const", bufs=1) as cst, \
             tc.tile_pool(name="xin", bufs=2) as xin, \
             tc.tile_pool(name="pers", bufs=1) as pers, \
             tc.tile_pool(name="ab", bufs=2) as abp, \
             tc.tile_pool(name="col", bufs=2) as colp, \
             tc.tile_pool(name="outp", bufs=6) as outp, \
             tc.tile_pool(name="dram", bufs=1, space="DRAM") as dram, \
             tc.tile_pool(name="ps_s", bufs=4, space="PSUM") as ps_s, \
             tc.tile_pool(name="ps_o", bufs=2, space="PSUM") as ps_o, \
             tc.tile_pool(name="ps_r", bufs=1, space="PSUM") as ps_r:

            m2_sb = cst.tile([P, NT, W], F)
            nc.sync.dma_start(m2_sb[:], m2.rearrange("t p w -> p t w"))
            di_sb = cst.tile([P, P], H)
            nc.sync.dma_start(di_sb[:], di)
            ones_sb = cst.tile([P, P], F)
            nc.sync.dma_start(ones_sb[:], ones)

            # ---- stream shard in: squares + fp16 cast per token tile, then
            # xbar-transpose each cast tile for every level (2 DMA queues) ----
            xh = pers.tile([P, L, NT, D], H)       # fp16 shard, 6.3 MB
            xt_all = pers.tile([P, L, KD, N], H)   # fp16 X^T per level, 6.3 MB
            part = pers.tile([P, 2 * NT], F)       # per-half-tile sumsq partials
            HLF = L * D // 2
            for j in range(NT):
                xt_in = xin.tile([P, L * D], F, tag="x")
                nc.sync.dma_start(xt_in[:], xr[:, j, :, :])
                for h in range(2):
                    sq = xin.tile([P, HLF], F, tag="sq")
                    nc.scalar.activation(sq[:], xt_in[:, h * HLF:(h + 1) * HLF],
                                         AF.Square, bias=0.0, scale=1.0,
                                         accum_out=part[:, 2 * j + h:2 * j + h + 1])
                nc.vector.tensor_copy(
                    xh[:, :, j, :],
                    xt_in[:].rearrange("p (l d) -> p l d", l=L))
                for l in range(L):
                    eng = nc.sync if (j * L + l) % 2 == 0 else nc.scalar
                    eng.dma_start_transpose(
                        xt_all[:, l, :, j * P:(j + 1) * P], xh[:, l, j, :])

            # ---- global scale c = rsqrt(max(ssq, eps)) / sqrt(D) ----
            p128 = colp.tile([P, 1], F, tag="p128")
            nc.vector.reduce_sum(p128[:], part[:], axis=mybir.AxisListType.X)
            pr = ps_r.tile([P, 1], F)
            nc.tensor.matmul(pr[:], ones_sb[:], p128[:], start=True, stop=True)
            ss_sb = colp.tile([P, 1], F, tag="ss")
            nc.vector.tensor_copy(ss_sb[:], pr[:])
            cc_in = dram.tile([P, 1], F)
            cc_out = dram.tile([P, 1], F)
            nc.sync.dma_start(cc_in[:], ss_sb[:])
            nc.gpsimd.collective_compute(
                "AllReduce", ADD, replica_groups=[list(range(8))],
                ins=[cc_in.opt()], outs=[cc_out.opt()])
            gs = colp.tile([P, 1], F, tag="gs")
            nc.sync.dma_start(gs[:], cc_out[:])
            nc.vector.tensor_scalar_max(gs[:], gs[:], float(L2_EPS))
            lns = colp.tile([P, 1], F, tag="lns")
            nc.scalar.activation(lns[:], gs[:], AF.Ln, bias=0.0, scale=float(D))
            c_sb = colp.tile([P, 1], F, tag="c")
            nc.scalar.activation(c_sb[:], lns[:], AF.Exp, bias=0.0, scale=-0.5)

            # ---- attention per level ----
            for l in range(L):
                xt_sb = xt_all[:, l]   # [P, KD, N] fp16 X^T for this level
                a_sb = abp.tile([P, NT, W], H, tag="a")
                dsum = colp.tile([P, NT], F, tag="dsum")
                for t in range(NT):
                    lo, hi = max(t - 1, 0), min(t + 2, NT)
                    c0, c1 = (lo - t + 1) * P, (hi - t + 1) * P
                    s_ps = ps_s.tile([P, W], F, tag="sps")
                    for kd in range(KD):
                        nc.tensor.matmul(s_ps[:, c0:c1],
                                         xt_sb[:, kd, t * P:(t + 1) * P],
                                         xt_sb[:, kd, lo * P:hi * P],
                                         start=(kd == 0), stop=(kd == KD - 1))
                    nc.vector.tensor_tensor(s_ps[:, c0:c1], s_ps[:, c0:c1],
                                            m2_sb[:, t, c0:c1], ADD)
                    nc.scalar.activation(a_sb[:, t, c0:c1], s_ps[:, c0:c1], AF.Exp,
                                         bias=0.0, scale=c_sb[:],
                                         accum_out=dsum[:, t:t + 1])
                    # self-attention weight onto the diagonal of block (t, t)
                    nc.vector.tensor_tensor(a_sb[:, t, P:2 * P], a_sb[:, t, P:2 * P],
                                            di_sb[:], ADD)

                denom = colp.tile([P, NT], F, tag="den")
                nc.vector.tensor_scalar_add(denom[:], dsum[:], SELF_W)
                recip = colp.tile([P, NT], F, tag="rec")
                nc.vector.reciprocal(recip[:], denom[:])

                for t in range(NT):
                    o_ps = ps_o.tile([P, D], F, tag="ops")
                    ks = [k for k in (t - 1, t, t + 1) if 0 <= k < NT]
                    for r, k in enumerate(ks):
                        # block(k, t): A rows of query-tile k, key-tile t
                        nc.tensor.matmul(o_ps[:],
                                         a_sb[:, k, (t - k + 1) * P:(t - k + 2) * P],
                                         xh[:, l, k, :],
                                         start=(r == 0), stop=(r == len(ks) - 1))
                    out_sb = outp.tile([P, D], F, tag="o")
                    if t % 2 == 0:
                        nc.scalar.activation(out_sb[:], o_ps[:], AF.Copy,
                                             bias=0.0, scale=recip[:, t:t + 1])
                    else:
                        nc.vector.tensor_scalar_mul(out_sb[:], o_ps[:],
                                                    recip[:, t:t + 1])
                    nc.sync.dma_start(orr[:, t, l, :], out_sb[:])
    nc.compile()
    return nc


def _get_nc():
    if "nc" not in _cache:
        _cache["nc"] = _build_nc()
    return _cache["nc"]


def _consts():
    if "consts" not in _cache:
        _cache["consts"] = {
            "m2": _masks(),
            "di": (np.float16(SELF_W) * np.eye(P)).astype(np.float16),
            "ones": np.ones((P, P), np.float32),
        }
    return _cache["consts"]


def _install_ntff_hook():
    """The agent image's antenv package lacks axon_hooks; recreate the NTFF
    profile hook (ctypes into libaxon_pjrt.so) and register it so
    run_bass_kernel_spmd(trace=True) can capture profiles. Only used by the
    local test harness (KERNEL_TRACE=1); never on the default path."""
    if _cache.get("hook_installed"):
        return
    import contextlib
    import ctypes
    import sys
    import types

    so_path = "/opt/axon/libaxon_pjrt.so"
    lib = ctypes.CDLL(so_path)
    lib.axon_start_nrt_profile.argtypes = [ctypes.POINTER(ctypes.c_int64), ctypes.c_size_t]
    lib.axon_start_nrt_profile.restype = ctypes.c_int64
    lib.axon_stop_nrt_profile.argtypes = [ctypes.c_char_p]
    lib.axon_stop_nrt_profile.restype = ctypes.c_int64

    @contextlib.contextmanager
    def _hook(output_dir, device_ids):
        import jax
        jax.devices()
        if device_ids:
            ids = (ctypes.c_int64 * len(device_ids))(*device_ids)
            rc = lib.axon_start_nrt_profile(ids, len(device_ids))
        else:
            rc = lib.axon_start_nrt_profile(None, 0)
        if rc != 0:
            raise RuntimeError(f"axon_start_nrt_profile rc={rc}")
        try:
            yield
        finally:
            n = lib.axon_stop_nrt_profile(str(output_dir).encode())
            print(f"ntff profile: {n} file(s) written to {output_dir}", file=sys.stderr)

    mod = types.ModuleType("antenv.axon_hooks")
    mod.get_axon_ntff_profile_hook = lambda: _hook
    mod.set_axon_ntff_profile_hook = lambda h: None
    import antenv
    antenv.axon_hooks = mod
    sys.modules["antenv.axon_hooks"] = mod
    _cache["hook_installed"] = True


last_exec_time_ns = {"norm": None, "attn": None}


def kernel(levels: np.ndarray) -> np.ndarray:
    from concourse.bass_utils import run_bass_kernel_spmd

    assert levels.shape == (B, N, L, D) and levels.dtype == np.float32
    nc = _get_nc()
    trace = os.environ.get("KERNEL_TRACE", "0") == "1"
    if trace:
        try:
            _install_ntff_hook()
        except Exception as e:
            print(f"ntff hook unavailable ({e}); tracing disabled")
            trace = False
    cores = list(range(8))

    consts = _consts()
    in_maps = [{"x": np.ascontiguousarray(levels[b]), "m2": consts["m2"],
                "di": consts["di"], "ones": consts["ones"]} for b in range(B)]
    if trace:
        try:
            res = run_bass_kernel_spmd(nc, in_maps, core_ids=cores, trace=True)
        except Exception as e:
            print(f"traced run failed ({e}); retrying untraced")
            res = run_bass_kernel_spmd(nc, in_maps, core_ids=cores)
    else:
        res = run_bass_kernel_spmd(nc, in_maps, core_ids=cores)
    last_exec_time_ns["attn"] = res.exec_time_ns
    last_exec_time_ns["norm"] = 0

    return np.stack([r["o"] for r in res.results], axis=0)
